# revision 4
# baseline (speedup 1.0000x reference)
"""Bass/Trainium2 kernel for batched multi-head self-attention.

Module math (per batch b):
    q = vec @ Wq; k = vec @ Wk; v = vec @ Wv            (per head h, dim d=16)
    S = q k^T / sqrt(d);  P = softmax_j(S);  recv = P v
    out = recv @ Wo

Sharding: data-parallel over batch (8 batches -> 8 NeuronCores), weights
replicated. Each core runs an identical Bass program on its vec slice.

Shapes (hardcoded): vec [8, 1024, 128]; Wq/Wk/Wv [128, 8, 16]; Wo [8, 16, 128].
"""

import sys

sys.path.insert(0, "/opt/trn_rl_repo")

from contextlib import ExitStack

import numpy as np

import concourse.bacc as bacc
import concourse.tile as tile
from concourse import mybir
from concourse.bass_utils import run_bass_kernel_spmd
from concourse.masks import make_identity

F32 = mybir.dt.float32
Exp = mybir.ActivationFunctionType.Exp

B, N, X, H, D = 8, 1024, 128, 8, 16
NCHUNK = N // 128          # 8 chunks of 128 along the token dim
SCALE = 0.25               # 1/sqrt(16)

_CACHED_NC = None


def build_nc():
    """Build the per-core Bass program (identical on all cores)."""
    nc = bacc.Bacc("TRN2")

    # DRAM I/O. Weight tensors arrive pre-permuted from numpy (see kernel()).
    d_wq = [nc.dram_tensor(f"wq{r}", (X, 128), F32, kind="ExternalInput")
            for r in range(3)]
    d_wk = [nc.dram_tensor(f"wk{r}", (X, 128), F32, kind="ExternalInput")
            for r in range(3)]
    d_wv = nc.dram_tensor("wv", (X, 128), F32, kind="ExternalInput")
    d_wo = nc.dram_tensor("wo", (128, X), F32, kind="ExternalInput")
    d_vec = nc.dram_tensor("vec", (N, X), F32, kind="ExternalInput")
    d_e8 = nc.dram_tensor("e8c", (H, 128), F32, kind="ExternalInput")
    d_ones = nc.dram_tensor("ones", (1, N), F32, kind="ExternalInput")
    d_out = nc.dram_tensor("out", (N, X), F32, kind="ExternalOutput")

    with tile.TileContext(nc) as tc, ExitStack() as top:
        const = top.enter_context(tc.tile_pool(name="const", bufs=1))
        ident = const.tile([128, 128], F32)
        make_identity(nc, ident)

        # Persistent SBUF tensors.
        w_sb = {}
        for name, dram in ([(f"wq{r}", d_wq[r]) for r in range(3)]
                           + [(f"wk{r}", d_wk[r]) for r in range(3)]
                           + [("wv", d_wv), ("wo", d_wo)]):
            t = const.tile([128, 128], F32, tag=f"w_{name}", name=f"w_{name}")
            nc.sync.dma_start(out=t[:], in_=dram[:, :])
            w_sb[name] = t

        vecT = const.tile([128, N], F32, tag="vecT")      # [x, n]
        # QT/KT layout: strip t=h%4 occupies partitions [32t, 32t+17):
        # rows 32t+d hold head-h dim d, row 32t+16 is the aug row
        # (ones for KT, -rowmax for QT).
        QT = {r: const.tile([128, N], F32, tag=f"qt{r}", name=f"qt{r}")
              for r in range(3)}
        KT = {r: const.tile([128, N], F32, tag=f"kt{r}", name=f"kt{r}")
              for r in range(3)}
        # V layout: [128 j-in-chunk, jc, 17*h + d], col 17h+16 = ones.
        V_sb = const.tile([128, NCHUNK, 17 * H], F32, tag="vsb")
        # P^T per head: [128 j-in-chunk, jc*1024 + i], fp32.
        pt_pool = top.enter_context(tc.tile_pool(name="pt", bufs=2))
        # raw recv output (incl. den rows), per half of the heads
        raw = {r: const.tile([128, N], F32, tag=f"raw{r}", name=f"raw{r}")
               for r in range(3)}
        recvT = const.tile([128, N], F32, tag="recvT")     # [(h d), i]
        recvN = const.tile([128, N], F32, tag="recvN")     # normalized
        den_sb = const.tile([H, N], F32, tag="den")
        rden = const.tile([H, N], F32, tag="rden")
        e8 = const.tile([H, 128], F32, tag="e8")           # expand matrix
        mha_sb = const.tile([128, NCHUNK, X], F32, tag="mha")

        nc.sync.dma_start(out=e8[:], in_=d_e8[:, :])
        # ones columns of V
        v_heads = V_sb[:].rearrange("p c (h s) -> p c h s", h=H)
        nc.vector.memset(v_heads[:, :, :, 16:17], 1.0)

        # ---- Phase 0: vecT via PE transposes; projections. ----
        with tc.tile_pool(name="stage", bufs=3) as stage, \
                tc.tile_pool(name="ps0", bufs=2, space="PSUM") as ps0, \
                tc.tile_pool(name="ps0b", bufs=2, space="PSUM") as ps0b:
            for c in range(NCHUNK):
                vt = stage.tile([128, 128], F32, tag="vstage")
                nc.sync.dma_start(out=vt[:], in_=d_vec[c * 128:(c + 1) * 128, :])
                pt_ = ps0b.tile([128, 128], F32, tag="trp")
                nc.tensor.transpose(pt_[:, :], vt[:], ident[:])
                nc.scalar.copy(vecT[:, c * 128:(c + 1) * 128], pt_[:, :])

            # QT/KT projections: psum = W.T @ vecT  -> [hd-pos, n]
            for rnd in range(3):
                for wname, dst in ((f"wq{rnd}", QT[rnd]), (f"wk{rnd}", KT[rnd])):
                    p = ps0.tile([128, N], F32, tag="proj")
                    for half in range(2):
                        sl = slice(half * 512, (half + 1) * 512)
                        nc.tensor.matmul(p[:, sl], w_sb[wname][:],
                                         vecT[:, sl], start=True, stop=True)
                    nc.scalar.copy(dst[:, :], p[:, :])
            # ones rows of KT
            for rnd in range(3):
                for t in range(3):
                    nc.sync.dma_start(
                        out=KT[rnd][32 * t + 16:32 * t + 17, :],
                        in_=d_ones[:, :])

            # V projection: per chunk [j, hd] = vecT[:,chunk].T @ Wv
            for c in range(NCHUNK):
                pv = ps0b.tile([128, 128], F32, tag="trp")
                nc.tensor.matmul(pv[:, :], vecT[:, c * 128:(c + 1) * 128],
                                 w_sb["wv"][:], start=True, stop=True)
                dst = V_sb[:, c, :].rearrange("p (h s) -> p h s", h=H)
                src = pv[:, :].rearrange("p (h d) -> p h d", h=H)
                nc.vector.tensor_copy(dst[:, :, 0:16], src[:])

        # ---- Main loop over heads. ----
        with tc.tile_pool(name="small", bufs=3) as small, \
                tc.tile_pool(name="psm", bufs=3, space="PSUM") as psm, \
                tc.tile_pool(name="psr", bufs=2, space="PSUM") as psr:
            for h in range(H):
                rnd, t = h // 3, h % 3
                sp = 32 * t
                qt, kt = QT[rnd], KT[rnd]

                # form1: S[i, j] row-tiled (K=16); row-max -> m_h.
                m_h = small.tile([128, NCHUNK], F32, tag="mh")
                for c in range(NCHUNK):
                    f1 = psm.tile([128, N], F32, tag="big")
                    for half in range(2):
                        sl = slice(half * 512, (half + 1) * 512)
                        nc.tensor.matmul(
                            f1[:, sl],
                            qt[sp:sp + 16, c * 128:(c + 1) * 128],
                            kt[sp:sp + 16, sl], start=True, stop=True)
                    nc.vector.tensor_reduce(
                        m_h[:, c:c + 1], f1[:, :], axis=mybir.AxisListType.X,
                        op=mybir.AluOpType.max, negate=True)

                # m-dance: [128, 8] -> transpose -> [8, 128] -> flatten
                # into the aug row of QT (value = -rowmax).
                trp = psr.tile([128, 512], F32, tag="recv")
                nc.tensor.transpose(trp[0:NCHUNK, 0:128], m_h[:], ident[:])
                m8 = small.tile([NCHUNK, 128], F32, tag="m8")
                nc.vector.tensor_copy(m8[:], trp[0:NCHUNK, 0:128])
                nc.sync.dma_start(out=qt[sp + 16:sp + 17, :], in_=m8[:])

                # S'^T tiles (K=17 augmented) + exp -> PT.
                PT = pt_pool.tile([128, NCHUNK * N], F32, tag="pt")
                for jc in range(NCHUNK):
                    st = psm.tile([128, N], F32, tag="big")
                    for half in range(2):
                        sl = slice(half * 512, (half + 1) * 512)
                        nc.tensor.matmul(
                            st[:, sl],
                            kt[sp:sp + 17, jc * 128:(jc + 1) * 128],
                            qt[sp:sp + 17, sl], start=True, stop=True)
                    nc.scalar.activation(PT[:, jc * N:jc * N + N], st[:, :],
                                         Exp, bias=0.0, scale=SCALE)

                # PV: recvT_aug[17, i] accumulated over j chunks.
                # Head h -> raw[h//3] rows [32*(h%3), +17), col-tiled.
                rv = raw[h // 3]
                for half in range(2):
                    prv = psr.tile([128, 512], F32, tag="recv")
                    for jc in range(NCHUNK):
                        nc.tensor.matmul(
                            prv[sp:sp + 17, :],
                            V_sb[:, jc, 17 * h:17 * h + 17],
                            PT[:, jc * N + half * 512: jc * N + (half + 1) * 512],
                            start=(jc == 0), stop=(jc == NCHUNK - 1))
                    nc.vector.tensor_copy(
                        rv[sp:sp + 17, half * 512:(half + 1) * 512],
                        prv[sp:sp + 17, :])

        # ---- Tail: normalize + output projection. ----
        with tc.tile_pool(name="pst", bufs=2, space="PSUM") as pst, \
                tc.tile_pool(name="pstb", bufs=2, space="PSUM") as pstb:
            # Remap raw -> recvT rows (16h+d) and den rows -> den_sb.
            for h in range(H):
                rv, sp = raw[h // 3], 32 * (h % 3)
                nc.sync.dma_start(out=recvT[16 * h:16 * h + 16, :],
                                  in_=rv[sp:sp + 16, :])
                nc.sync.dma_start(out=den_sb[h:h + 1, :],
                                  in_=rv[sp + 16:sp + 17, :])
            nc.vector.reciprocal(rden[:], den_sb[:])
            pe_ = pst.tile([128, N], F32, tag="expand")
            for half in range(2):
                sl = slice(half * 512, (half + 1) * 512)
                nc.tensor.matmul(pe_[:, sl], e8[:], rden[:, sl],
                                 start=True, stop=True)
            nc.vector.tensor_mul(recvN[:], recvT[:], pe_[:, :])
            for c in range(NCHUNK):
                po = pstb.tile([128, 128], F32, tag="mha")
                nc.tensor.matmul(po[:, :], recvN[:, c * 128:(c + 1) * 128],
                                 w_sb["wo"][:], start=True, stop=True)
                nc.scalar.copy(mha_sb[:, c, :], po[:, :])
                nc.sync.dma_start(out=d_out[c * 128:(c + 1) * 128, :],
                                  in_=mha_sb[:, c, :])

    nc.finalize()
    return nc


def _permute_weights(Wq, Wk, Wv, Wo):
    """Numpy-side weight layout prep."""
    def strip_pack(W, heads):
        # W [x, 8, 16] -> [x, 128]: cols 32t+d = W[:, heads[t], d], rest 0.
        out = np.zeros((X, 128), dtype=np.float32)
        for t, h in enumerate(heads):
            out[:, 32 * t:32 * t + 16] = W[:, h, :]
        return out

    rounds = [[0, 1, 2], [3, 4, 5], [6, 7]]
    e8c = np.zeros((H, 128), dtype=np.float32)
    for h in range(H):
        e8c[h, 16 * h:16 * h + 16] = 1.0
    d = dict(
        wv=np.ascontiguousarray(Wv.reshape(X, 128)),
        wo=np.ascontiguousarray(Wo.reshape(128, X)),
        e8c=e8c, ones=np.ones((1, N), dtype=np.float32),
    )
    for r, heads in enumerate(rounds):
        d[f"wq{r}"] = strip_pack(Wq, heads)
        d[f"wk{r}"] = strip_pack(Wk, heads)
    return d


def kernel(Wq, Wk, Wv, Wo, vec, trace=False):
    global _CACHED_NC
    if _CACHED_NC is None:
        _CACHED_NC = build_nc()
    nc = _CACHED_NC

    w = _permute_weights(np.asarray(Wq, np.float32), np.asarray(Wk, np.float32),
                         np.asarray(Wv, np.float32), np.asarray(Wo, np.float32))
    vec = np.asarray(vec, np.float32)
    in_maps = [dict(w, vec=np.ascontiguousarray(vec[b])) for b in range(B)]
    res = run_bass_kernel_spmd(nc, in_maps, core_ids=list(range(B)),
                               trace=trace)
    out = np.stack([res.results[b]["out"] for b in range(B)])
    if trace:
        return out, res
    return out


# revision 7
# speedup vs baseline: 1.4558x; 1.4558x over previous
"""Bass/Trainium2 kernel for batched multi-head self-attention.

Module math (per batch b):
    q = vec @ Wq; k = vec @ Wk; v = vec @ Wv            (per head h, dim d=16)
    S = q k^T / sqrt(d);  P = softmax_j(S);  recv = P v
    out = recv @ Wo

Sharding: data-parallel over batch (8 batches -> 8 NeuronCores), weights
replicated. Each core runs an identical Bass program on its vec slice.

Shapes (hardcoded): vec [8, 1024, 128]; Wq/Wk/Wv [128, 8, 16]; Wo [8, 16, 128].
"""

import sys

sys.path.insert(0, "/opt/trn_rl_repo")

from contextlib import ExitStack

import numpy as np

import concourse.bacc as bacc
import concourse.tile as tile
from concourse import mybir
from concourse.bass_utils import run_bass_kernel_spmd
from concourse.masks import make_identity

F32 = mybir.dt.float32
F16 = mybir.dt.float16
BF16 = mybir.dt.bfloat16
Exp = mybir.ActivationFunctionType.Exp

B, N, X, H, D = 8, 1024, 128, 8, 16
NCHUNK = N // 128          # 8 chunks of 128 along the token dim
SCALE = 0.25               # 1/sqrt(16)

_CACHED_NC = None


def build_nc():
    """Build the per-core Bass program (identical on all cores)."""
    nc = bacc.Bacc("TRN2")

    # DRAM I/O. Weight tensors arrive pre-permuted from numpy (see kernel()).
    d_wq = [nc.dram_tensor(f"wq{r}", (X, 128), F32, kind="ExternalInput")
            for r in range(3)]
    d_wk = [nc.dram_tensor(f"wk{r}", (X, 128), F32, kind="ExternalInput")
            for r in range(3)]
    d_wv = nc.dram_tensor("wv", (X, 128), F32, kind="ExternalInput")
    d_wo = nc.dram_tensor("wo", (128, X), F32, kind="ExternalInput")
    d_vec = nc.dram_tensor("vec", (N, X), F32, kind="ExternalInput")
    d_e8 = nc.dram_tensor("e8c", (H, 128), F32, kind="ExternalInput")
    d_ones = nc.dram_tensor("ones", (1, N), F32, kind="ExternalInput")
    d_out = nc.dram_tensor("out", (N, X), F32, kind="ExternalOutput")

    with tile.TileContext(nc) as tc, ExitStack() as top:
        const = top.enter_context(tc.tile_pool(name="const", bufs=1))
        ident = const.tile([128, 128], F32)
        make_identity(nc, ident)

        # Persistent SBUF tensors.
        w_sb = {}
        for name, dram in ([(f"wq{r}", d_wq[r]) for r in range(3)]
                           + [(f"wk{r}", d_wk[r]) for r in range(3)]
                           + [("wv", d_wv), ("wo", d_wo)]):
            t = const.tile([128, 128], F32, tag=f"w_{name}", name=f"w_{name}")
            nc.sync.dma_start(out=t[:], in_=dram[:, :])
            w_sb[name] = t

        vecT = const.tile([128, N], F32, tag="vecT")      # [x, n]
        # QT/KT layout: strip t=h%4 occupies partitions [32t, 32t+17):
        # rows 32t+d hold head-h dim d, row 32t+16 is the aug row
        # (ones for KT, -rowmax for QT).
        QT = {r: const.tile([128, N], F32, tag=f"qt{r}", name=f"qt{r}")
              for r in range(3)}
        KT = {r: const.tile([128, N], F32, tag=f"kt{r}", name=f"kt{r}")
              for r in range(3)}
        QTh = {r: const.tile([128, N], BF16, tag=f"qth{r}", name=f"qth{r}")
               for r in range(3)}
        KTh = {r: const.tile([128, N], BF16, tag=f"kth{r}", name=f"kth{r}")
               for r in range(3)}
        # V layout: [128 j-in-chunk, jc, 17*h + d], col 17h+16 = ones.
        V_sb = const.tile([128, NCHUNK, 17 * H], F16, tag="vsb")
        # P^T per head: [128 j-in-chunk, jc*1024 + i], fp32.
        pt_pool = top.enter_context(tc.tile_pool(name="pt", bufs=2))
        # raw recv output (incl. den rows), per half of the heads
        raw = {r: const.tile([128, N], F32, tag=f"raw{r}", name=f"raw{r}")
               for r in range(3)}
        recvT = const.tile([128, N], F32, tag="recvT")     # [(h d), i]
        recvN = const.tile([128, N], F32, tag="recvN")     # normalized
        den_sb = const.tile([H, N], F32, tag="den")
        rden = const.tile([H, N], F32, tag="rden")
        e8 = const.tile([H, 128], F32, tag="e8")           # expand matrix
        mha_sb = const.tile([128, NCHUNK, X], F32, tag="mha")

        nc.sync.dma_start(out=e8[:], in_=d_e8[:, :])
        # ones columns of V
        v_heads = V_sb[:].rearrange("p c (h s) -> p c h s", h=H)
        nc.vector.memset(v_heads[:, :, :, 16:17], 1.0)

        # ---- Phase 0: vecT via PE transposes; projections. ----
        with tc.tile_pool(name="stage", bufs=3) as stage, \
                tc.tile_pool(name="ps0", bufs=2, space="PSUM") as ps0, \
                tc.tile_pool(name="ps0b", bufs=2, space="PSUM") as ps0b:
            for c in range(NCHUNK):
                vt = stage.tile([128, 128], F32, tag="vstage")
                nc.sync.dma_start(out=vt[:], in_=d_vec[c * 128:(c + 1) * 128, :])
                pt_ = ps0b.tile([128, 128], F32, tag="trp")
                nc.tensor.transpose(pt_[:, :], vt[:], ident[:])
                nc.scalar.copy(vecT[:, c * 128:(c + 1) * 128], pt_[:, :])

            # QT/KT projections: psum = W.T @ vecT  -> [hd-pos, n]
            for rnd in range(3):
                for wname, dst, dsth in ((f"wq{rnd}", QT[rnd], QTh[rnd]),
                                         (f"wk{rnd}", KT[rnd], KTh[rnd])):
                    p = ps0.tile([128, N], F32, tag="proj")
                    for half in range(2):
                        sl = slice(half * 512, (half + 1) * 512)
                        nc.tensor.matmul(p[:, sl], w_sb[wname][:],
                                         vecT[:, sl], start=True, stop=True)
                    nc.scalar.copy(dst[:, :], p[:, :])
                    nc.vector.tensor_copy(dsth[:, :], p[:, :])
            # ones rows of KT
            for rnd in range(3):
                for t in range(3):
                    nc.sync.dma_start(
                        out=KT[rnd][32 * t + 16:32 * t + 17, :],
                        in_=d_ones[:, :])

            # V projection: per chunk [j, hd] = vecT[:,chunk].T @ Wv
            for c in range(NCHUNK):
                pv = ps0b.tile([128, 128], F32, tag="trp")
                nc.tensor.matmul(pv[:, :], vecT[:, c * 128:(c + 1) * 128],
                                 w_sb["wv"][:], start=True, stop=True)
                dst = V_sb[:, c, :].rearrange("p (h s) -> p h s", h=H)
                src = pv[:, :].rearrange("p (h d) -> p h d", h=H)
                nc.vector.tensor_copy(dst[:, :, 0:16], src[:])

        # ---- Main loop over heads. ----
        with tc.tile_pool(name="small", bufs=3) as small, \
                tc.tile_pool(name="psm", bufs=3, space="PSUM") as psm, \
                tc.tile_pool(name="psr", bufs=2, space="PSUM") as psr:
            for h in range(H):
                rnd, t = h // 3, h % 3
                sp = 32 * t
                qt, kt = QT[rnd], KT[rnd]

                # form1: S[i, j] row-tiled (K=16, fp16); row-max -> m_h.
                qth, kth = QTh[rnd], KTh[rnd]
                m_h = small.tile([128, NCHUNK], F32, tag="mh")
                for c in range(NCHUNK):
                    f1 = psm.tile([128, N], F32, tag="big")
                    for half in range(2):
                        sl = slice(half * 512, (half + 1) * 512)
                        nc.tensor.matmul(
                            f1[:, sl],
                            qth[sp:sp + 16, c * 128:(c + 1) * 128],
                            kth[sp:sp + 16, sl], start=True, stop=True)
                    nc.vector.tensor_reduce(
                        m_h[:, c:c + 1], f1[:, :], axis=mybir.AxisListType.X,
                        op=mybir.AluOpType.max, negate=True)

                # m-dance: [128, 8] -> transpose -> [8, 128] -> flatten
                # into the aug row of QT (value = -rowmax).
                trp = psr.tile([128, 512], F32, tag="recv")
                nc.tensor.transpose(trp[0:NCHUNK, 0:128], m_h[:], ident[:])
                m8 = small.tile([NCHUNK, 128], F32, tag="m8")
                nc.vector.tensor_copy(m8[:], trp[0:NCHUNK, 0:128])
                nc.sync.dma_start(out=qt[sp + 16:sp + 17, :], in_=m8[:])

                # S'^T tiles (K=17 augmented) + exp -> PT.
                PT = pt_pool.tile([128, NCHUNK * N], F16, tag="pt")
                for jc in range(NCHUNK):
                    st = psm.tile([128, N], F32, tag="big")
                    for half in range(2):
                        sl = slice(half * 512, (half + 1) * 512)
                        nc.tensor.matmul(
                            st[:, sl],
                            kt[sp:sp + 17, jc * 128:(jc + 1) * 128],
                            qt[sp:sp + 17, sl], start=True, stop=True)
                    nc.scalar.activation(PT[:, jc * N:jc * N + N], st[:, :],
                                         Exp, bias=0.0, scale=SCALE)

                # PV: recvT_aug[17, i] accumulated over j chunks.
                # Head h -> raw[h//3] rows [32*(h%3), +17), col-tiled.
                rv = raw[h // 3]
                for half in range(2):
                    prv = psr.tile([128, 512], F32, tag="recv")
                    for jc in range(NCHUNK):
                        nc.tensor.matmul(
                            prv[sp:sp + 17, :],
                            V_sb[:, jc, 17 * h:17 * h + 17],
                            PT[:, jc * N + half * 512: jc * N + (half + 1) * 512],
                            start=(jc == 0), stop=(jc == NCHUNK - 1))
                    nc.vector.tensor_copy(
                        rv[sp:sp + 17, half * 512:(half + 1) * 512],
                        prv[sp:sp + 17, :])

        # ---- Tail: normalize + output projection. ----
        with tc.tile_pool(name="pst", bufs=2, space="PSUM") as pst, \
                tc.tile_pool(name="pstb", bufs=2, space="PSUM") as pstb:
            # Remap raw -> recvT rows (16h+d) and den rows -> den_sb.
            for h in range(H):
                rv, sp = raw[h // 3], 32 * (h % 3)
                nc.sync.dma_start(out=recvT[16 * h:16 * h + 16, :],
                                  in_=rv[sp:sp + 16, :])
                nc.sync.dma_start(out=den_sb[h:h + 1, :],
                                  in_=rv[sp + 16:sp + 17, :])
            nc.vector.reciprocal(rden[:], den_sb[:])
            pe_ = pst.tile([128, N], F32, tag="expand")
            for half in range(2):
                sl = slice(half * 512, (half + 1) * 512)
                nc.tensor.matmul(pe_[:, sl], e8[:], rden[:, sl],
                                 start=True, stop=True)
            nc.vector.tensor_mul(recvN[:], recvT[:], pe_[:, :])
            for c in range(NCHUNK):
                po = pstb.tile([128, 128], F32, tag="mha")
                nc.tensor.matmul(po[:, :], recvN[:, c * 128:(c + 1) * 128],
                                 w_sb["wo"][:], start=True, stop=True)
                nc.scalar.copy(mha_sb[:, c, :], po[:, :])
                nc.sync.dma_start(out=d_out[c * 128:(c + 1) * 128, :],
                                  in_=mha_sb[:, c, :])

    nc.finalize()
    return nc


def _permute_weights(Wq, Wk, Wv, Wo):
    """Numpy-side weight layout prep."""
    def strip_pack(W, heads):
        # W [x, 8, 16] -> [x, 128]: cols 32t+d = W[:, heads[t], d], rest 0.
        out = np.zeros((X, 128), dtype=np.float32)
        for t, h in enumerate(heads):
            out[:, 32 * t:32 * t + 16] = W[:, h, :]
        return out

    rounds = [[0, 1, 2], [3, 4, 5], [6, 7]]
    e8c = np.zeros((H, 128), dtype=np.float32)
    for h in range(H):
        e8c[h, 16 * h:16 * h + 16] = 1.0
    d = dict(
        wv=np.ascontiguousarray(Wv.reshape(X, 128)),
        wo=np.ascontiguousarray(Wo.reshape(128, X)),
        e8c=e8c, ones=np.ones((1, N), dtype=np.float32),
    )
    for r, heads in enumerate(rounds):
        d[f"wq{r}"] = strip_pack(Wq, heads)
        d[f"wk{r}"] = strip_pack(Wk, heads)
    return d


def kernel(Wq, Wk, Wv, Wo, vec, trace=False):
    global _CACHED_NC
    if _CACHED_NC is None:
        _CACHED_NC = build_nc()
    nc = _CACHED_NC

    w = _permute_weights(np.asarray(Wq, np.float32), np.asarray(Wk, np.float32),
                         np.asarray(Wv, np.float32), np.asarray(Wo, np.float32))
    vec = np.asarray(vec, np.float32)
    in_maps = [dict(w, vec=np.ascontiguousarray(vec[b])) for b in range(B)]
    res = run_bass_kernel_spmd(nc, in_maps, core_ids=list(range(B)),
                               trace=trace)
    out = np.stack([res.results[b]["out"] for b in range(B)])
    if trace:
        return out, res
    return out
